# revision 1
# baseline (speedup 1.0000x reference)
"""Trainium2 Bass kernel for a BertPooler-style segment-reduce:

    first = h[:, 0, :]
    subj  = mean(h[b, subj_range[b,0]:subj_range[b,1], :])
    obj   = mean(h[b, obj_range[b,0]:obj_range[b,1], :])
    out   = tanh(concat([first, subj, obj]) @ W.T + b)

Strategy (8 NeuronCores, 4x2 grid: batch-groups x output-column-groups):
  - Core (bg, jg) owns 16 of the 64 batch rows and 512 of the 1024 output
    columns; W is sharded by output column so each core reads half of W.
  - Ranges cover <= 32 tokens, so instead of a full masked reduction over
    S=512 we indirect-DMA gather only the 32-token windows (plus the CLS
    rows) -- reading ~4 MB of hidden state per core instead of 32 MB.
  - Gather indices are built with one broadcast-source DMA (range rows
    replicated across partitions with a zero-stride AP) plus one integer
    vector add -- the shortest possible serial chain before the gathers.
  - Window sums/means are computed on the TensorEngine as masked matmuls
    whose masks are built on-device from the ranges; the reduction matmuls
    directly produce the transposed feature matrix featsT[3072, BL].
  - The pooler matmul streams the W^T shard (host-pretransposed, loaded in
    4 large DMAs across both HWDGE rings) through the PE as the moving
    fp32r operand; bias is folded in as a K=1 accumulating matmul; tanh
    runs on the scalar engine.
  - All small constants ride in one packed [128, 576] tensor (single DMA)
    to amortize per-DMA fixed costs.
"""

import numpy as np

B, S, H = 64, 512, 1024
N_CORES = 8
GJ = 2                     # output-column groups (W shard factor)
GB = N_CORES // GJ         # batch groups
BL = B // GB               # batches per core
NG = BL // 4               # 4-batch gather groups per core
NGT = 2 * NG               # gather count (groups x {subj, obj})
JW = H // GJ               # output columns per core
K3 = 3 * H                 # feats dim
P = 128
NKT = K3 // P              # 24 contraction tiles
NWC = 4                    # W chunks (NKT/NWC k-tiles per DMA)
WMAX = 32                  # max range length the fast path supports

# packed-constant column map (f32 columns; int32 stored as bit patterns)
RNG_C0, RNG_C1 = 0, 16            # rows 0:4 -> rng4 [4, 16]
BASE_C0, BASE_C1 = 16, 24         # baseg8 int32 [128, 8]
JV_C = 24                         # jvec f32 [128, 1]
IB_C0, IB_C1 = 25, 25 + BL        # identity [BL, BL]
ONE_C0, ONE_C1 = 41, 41 + BL      # ones row [1, BL]
BIAS_C0, BIAS_C1 = 57, 57 + JW    # bias [1, JW]
SEL_C0, SEL_C1 = BIAS_C1, BIAS_C1 + 4   # sel4 [128, 4]
JVI_C = SEL_C1                    # jvec as int32 bits [128, 1]
CPK_W = JVI_C + 3                 # pad to 576

_cache: dict = {}


def _consts_cpk():
    """Data-independent part of the packed constants tensor."""
    p = np.arange(P)
    cpk = np.zeros((P, CPK_W), np.float32)
    baseg8 = np.empty((P, 8), np.int32)
    for c in range(8):
        g = c // 2
        baseg8[:, c] = (g * 4 + p // WMAX) * S + (p % WMAX)
    cpk[:, BASE_C0:BASE_C1] = baseg8.view(np.float32)
    cpk[:, JV_C] = (p % WMAX).astype(np.float32)
    cpk[0:BL, IB_C0:IB_C1] = np.eye(BL, dtype=np.float32)
    cpk[0, ONE_C0:ONE_C1] = 1.0
    cpk[:, SEL_C0:SEL_C1] = (p[:, None] // WMAX ==
                             np.arange(4)[None, :]).astype(np.float32)
    cpk[:, JVI_C] = (p % WMAX).astype(np.int32).view(np.float32)
    return cpk


def _build_fast(reps=1, hw_loop=False, skip_gather=False, skip_w=False,
                num_devices=None):
    import contextlib
    import concourse.bass as bass
    import concourse.tile as tile
    from concourse import bacc, mybir

    f32 = mybir.dt.float32
    f32r = mybir.dt.float32r
    i32 = mybir.dt.int32

    nc = bacc.Bacc("TRN2", target_bir_lowering=False, debug=False,
                   num_devices=num_devices or N_CORES)

    h = nc.dram_tensor("h", [BL * S, H], f32, kind="ExternalInput")
    cpk = nc.dram_tensor("cpk", [P, CPK_W], f32, kind="ExternalInput")
    wt = nc.dram_tensor("wt", [K3, JW], f32r, kind="ExternalInput")
    out = nc.dram_tensor("out", [BL, JW], f32, kind="ExternalOutput")

    NHF = JW // 512            # 512-wide moving-operand slices
    KTC = NKT // NWC           # k-tiles per W chunk

    with tile.TileContext(nc) as tc:
        with (
            tc.tile_pool(name="consts", bufs=1) as cpool,
            tc.tile_pool(name="work", bufs=1) as wpool,
            tc.tile_pool(name="wtiles", bufs=NWC) as wtpool,
            tc.tile_pool(name="psum", bufs=1, space="PSUM") as ppool,
        ):
          loop_ctx = (tc.For_i(0, reps, 1) if hw_loop
                      else contextlib.nullcontext())
          with loop_ctx:
            for _rep in range(1 if hw_loop else reps):
                # --- range rows broadcast to all partitions (1 DMA) ---
                bc = wpool.tile([P, 16], i32, tag="bc")
                nc.vector.memset(bc[:], 0)
                src = cpk.ap()[0:4, RNG_C0:RNG_C1].bitcast(i32)
                src = src[:, None, :].to_broadcast([4, P // 4, 16])
                nc.sync.dma_start(bc[:], src)

                # --- packed constants (1 DMA) ---
                cpk_t = cpool.tile([P, CPK_W], f32, tag="cpk")
                nc.sync.dma_start(cpk_t[:], cpk[:, :])
                jvec_t = cpk_t[:, JV_C:JV_C + 1]
                sel4_t = cpk_t[:, SEL_C0:SEL_C1]
                iB_t = cpk_t[0:BL, IB_C0:IB_C1]
                onesB_t = cpk_t[0:1, ONE_C0:ONE_C1]
                bias_t = cpk_t[0:1, BIAS_C0:BIAS_C1]
                baseg_t = cpk_t[:, BASE_C0:BASE_C1].bitcast(i32)

                # CLS rows: h[b*S, :] -- fixed-stride DMA on the ACT ring
                cls_t = wpool.tile([BL, H], f32, tag="cls")
                h_bsd = h.ap().rearrange("(b s) d -> b s d", s=S)
                nc.scalar.dma_start(cls_t[:], h_bsd[:, 0, :])

                # --- gather indices: one int add ---
                idx_i = wpool.tile([P, 8], i32, tag="idxi")
                nc.vector.tensor_add(idx_i[:], bc[:, 0:8], baseg_t)

                # --- window masks (off the gather critical path) ---
                masks = None
                if not skip_gather:
                    lens_i = wpool.tile([P, 8], i32, tag="lensi")
                    nc.vector.tensor_sub(lens_i[:], bc[:, 8:16], bc[:, 0:8])
                    lens_f = wpool.tile([P, 8], f32, tag="lensf")
                    nc.vector.tensor_copy(lens_f[:], lens_i[:])
                    cmp = wpool.tile([P, 8], f32, tag="cmp")
                    nc.vector.tensor_tensor(out=cmp[:],
                                            in0=jvec_t.to_broadcast([P, 8]),
                                            in1=lens_f[:],
                                            op=mybir.AluOpType.is_lt)
                    rcp = wpool.tile([P, 8], f32, tag="rcp")
                    nc.vector.reciprocal(rcp[:], lens_f[:])
                    nwt = wpool.tile([P, 8], f32, tag="nwt")
                    nc.vector.tensor_mul(nwt[:], lens_f[:], rcp[:])
                    nc.vector.tensor_scalar(out=nwt[:], in0=nwt[:],
                                            scalar1=-1.0, scalar2=2.0,
                                            op0=mybir.AluOpType.mult,
                                            op1=mybir.AluOpType.add)
                    nc.vector.tensor_mul(rcp[:], rcp[:], nwt[:])
                    u = wpool.tile([P, 8], f32, tag="u")
                    nc.vector.tensor_mul(u[:], cmp[:], rcp[:])
                    masks = []
                    for c in range(NGT):
                        m = wpool.tile([P, 4], f32, tag=f"mask{c}")
                        nc.vector.tensor_scalar_mul(m[:], sel4_t,
                                                    u[:, c:c + 1])
                        masks.append(m)

                # --- gather the 32-token windows ---
                gts = []
                for c in range(0 if skip_gather else NGT):
                    gt = wpool.tile([P, H], f32, tag=f"gt{c}")
                    gts.append(gt)
                    nc.gpsimd.indirect_dma_start(
                        out=gt[:], out_offset=None,
                        in_=h.ap(),
                        in_offset=bass.IndirectOffsetOnAxis(
                            ap=idx_i[:, c:c + 1], axis=0),
                        bounds_check=BL * S - 1,
                        oob_is_err=False,
                    )

                # --- W chunks: 4 big DMAs alternating HWDGE rings ---
                wcs = []
                wt_r = wt.ap().rearrange("(c t p) j -> c p t j",
                                         p=P, t=KTC)
                for c4 in range(0 if skip_w else NWC):
                    wc = wtpool.tile([P, KTC * JW], f32r, tag="wc")
                    eng = nc.sync if c4 % 2 == 0 else nc.scalar
                    eng.dma_start(
                        wc[:].rearrange("p (t j) -> p t j", t=KTC),
                        wt_r[c4])
                    wcs.append(wc)

                # --- reduction matmuls -> featsT[3072, BL] ---
                # ftp column = kt*BL + b ; kt = seg*8 + ks
                ftp = ppool.tile([P, NKT * BL], f32, tag="ftp", space="PSUM")
                if skip_gather:
                    nc.tensor.matmul(out=ftp[:, 0:NKT * BL],
                                     lhsT=cls_t[:BL, 0:P],
                                     rhs=cls_t[:BL, 0:NKT * BL],
                                     start=True, stop=True)
                for ks in range(0 if skip_gather else 8):
                    nc.tensor.matmul(out=ftp[:, ks * BL:(ks + 1) * BL],
                                     lhsT=cls_t[:BL, ks * P:(ks + 1) * P],
                                     rhs=iB_t, start=True, stop=True)
                if not skip_gather:
                    for c in range(NGT):
                        g, s = c // 2, c % 2
                        for ks in range(8):
                            kt = (1 + s) * 8 + ks
                            col = kt * BL + 4 * g
                            nc.tensor.matmul(
                                out=ftp[:, col:col + 4],
                                lhsT=gts[c][:, ks * P:(ks + 1) * P],
                                rhs=masks[c][:, :4], start=True, stop=True)
                ft_sb = wpool.tile([P, NKT * BL], f32r, tag="ftsb")
                # per-segment copies so the pooler matmul can start on
                # segment 0 (CLS) while subj/obj reductions are in flight
                for seg in range(3):
                    nc.vector.tensor_copy(
                        ft_sb[:, seg * 8 * BL:(seg + 1) * 8 * BL],
                        ftp[:, seg * 8 * BL:(seg + 1) * 8 * BL])

                # --- pooler matmul: out[BL, JW] = featsT.T @ W^T + bias ---
                op = []
                for hf in range(NHF):
                    op_t = ppool.tile([BL, 512], f32, tag=f"op{hf}",
                                      space="PSUM")
                    op.append(op_t)
                for kt in range(0 if skip_w else NKT):
                    c4, t = divmod(kt, KTC)
                    for hf in range(NHF):
                        nc.tensor.matmul(
                            out=op[hf][:BL, :],
                            lhsT=ft_sb[:, kt * BL:(kt + 1) * BL],
                            rhs=wcs[c4][:, t * JW + hf * 512:
                                        t * JW + (hf + 1) * 512],
                            start=(kt == 0), stop=False)
                o_sb = wpool.tile([BL, JW], f32, tag="osb")
                for hf in range(NHF):
                    nc.tensor.matmul(
                        out=op[hf][:BL, :],
                        lhsT=onesB_t,
                        rhs=bias_t[:1, hf * 512:(hf + 1) * 512],
                        start=skip_w, stop=True)
                    nc.scalar.activation(
                        out=o_sb[:BL, hf * 512:(hf + 1) * 512],
                        in_=op[hf][:BL, :],
                        func=mybir.ActivationFunctionType.Tanh)
                nc.sync.dma_start(out[:, :], o_sb[:])

    nc.compile()
    return nc


def _get_nc():
    if "nc" not in _cache:
        _cache["nc"] = _build_fast()
    return _cache["nc"]


def _core_inputs(hidden_states, subj, obj, wt_full, bias_full, consts, c):
    """Build the in_map for core c = bg * GJ + jg."""
    bg, jg = divmod(c, GJ)
    lo = bg * BL
    cpk = consts.copy()
    # rng4 [4, 16]: row q, cols 0:8 = starts, 8:16 = ends, order c=2g+s
    rng4 = np.empty((4, 16), np.int32)
    for g in range(NG):
        for q in range(4):
            bi = lo + 4 * g + q
            rng4[q, 2 * g] = subj[bi, 0]
            rng4[q, 2 * g + 1] = obj[bi, 0]
            rng4[q, 8 + 2 * g] = subj[bi, 1]
            rng4[q, 8 + 2 * g + 1] = obj[bi, 1]
    cpk[0:4, RNG_C0:RNG_C1] = rng4.view(np.float32)
    cpk[0, BIAS_C0:BIAS_C1] = bias_full[0, jg * JW:(jg + 1) * JW]
    return {
        "h": np.ascontiguousarray(hidden_states[lo:lo + BL].reshape(BL * S, H)),
        "cpk": cpk,
        "wt": np.ascontiguousarray(wt_full[:, jg * JW:(jg + 1) * JW]),
    }


def kernel(hidden_states, subj_range, obj_range, W, b):
    from concourse.bass_utils import run_bass_kernel_spmd

    hidden_states = np.asarray(hidden_states, dtype=np.float32)
    subj = np.asarray(subj_range).astype(np.int64)
    obj = np.asarray(obj_range).astype(np.int64)
    W = np.asarray(W, dtype=np.float32)
    b = np.asarray(b, dtype=np.float32)
    assert hidden_states.shape == (B, S, H)
    assert subj.shape == (B, 2) and obj.shape == (B, 2)

    max_len = max((subj[:, 1] - subj[:, 0]).max(), (obj[:, 1] - obj[:, 0]).max())
    assert max_len <= WMAX, "fast path requires range length <= 32"

    nc = _get_nc()
    consts = _consts_cpk()
    wt_full = np.ascontiguousarray(W.T)            # [3072, 1024]
    bias_full = np.ascontiguousarray(b[None, :])   # [1, 1024]

    in_maps = [_core_inputs(hidden_states, subj, obj, wt_full, bias_full,
                            consts, c) for c in range(N_CORES)]

    res = run_bass_kernel_spmd(nc, in_maps, core_ids=list(range(N_CORES)))
    out = np.empty((B, H), np.float32)
    for c in range(N_CORES):
        bg, jg = divmod(c, GJ)
        out[bg * BL:(bg + 1) * BL, jg * JW:(jg + 1) * JW] = res.results[c]["out"]
    return out



# revision 25
# speedup vs baseline: 2.1096x; 2.1096x over previous
"""Trainium2 Bass kernel for a BertPooler-style segment-reduce:

    first = h[:, 0, :]
    subj  = mean(h[b, subj_range[b,0]:subj_range[b,1], :])
    obj   = mean(h[b, obj_range[b,0]:obj_range[b,1], :])
    out   = tanh(concat([first, subj, obj]) @ W.T + b)

Strategy (8 NeuronCores, 4x2 grid: batch-groups x output-column-groups):
  - Core (bg, jg) owns 16 of the 64 batch rows and 512 of the 1024 output
    columns; W is sharded by output column so each core reads half of W.
  - Everything big rides in bf16 (tolerance is 2e-2): hidden state is cast
    on the host, W is cast + pre-arranged on the host so each W chunk DMA
    reads 6KB contiguous per partition.
  - Ranges cover <= 32 tokens; gather indices (start + lane offsets) and
    the 1/len reduction masks are host-precomputed (O(B) metadata, like
    the range packing itself), so the device does: load idx -> one big
    8-column indirect gather (split in 4 chunks to pipeline reductions).
  - Window sums/means are masked matmuls on the TensorEngine producing
    the transposed feature matrix featsT[3072, BL] directly; CLS rows are
    a strided DMA + identity-mask matmuls.
  - The pooler matmul streams the W shard through the PE as the moving
    bf16 operand; bias is a K=1 accumulating f32 matmul; tanh on ACT.
  - W chunk 0 streams concurrently with the gathers to keep the DMA
    engines saturated; chunks 1-3 are explicitly ordered after the last
    gather chunk (manual dep) so the gathers -- whose consumers are the
    long reduction tail -- are not starved by queue round-robin.
"""

import numpy as np

B, S, H = 64, 512, 1024
N_CORES = 8
GJ = 2                     # output-column groups (W shard factor)
GB = N_CORES // GJ         # batch groups
BL = B // GB               # batches per core
NG = BL // 4               # 4-batch gather groups per core
NGT = 2 * NG               # gather index columns (groups x {subj, obj})
JW = H // GJ               # output columns per core
K3 = 3 * H                 # feats dim
P = 128
NKT = K3 // P              # 24 contraction tiles
NWC = 4                    # W chunks
KTC = NKT // NWC           # k-tiles per W chunk
import os as _os
WMAX = 32                  # max range length the fast path supports
NW = 2 * BL                # windows per core (batches x {subj, obj})
# gather layout: TPP consecutive tokens per partition (one 2*TPP KB
# contiguous DRAM read per descriptor); a window spans PW partitions
TPP = int(_os.environ.get("K_TPP", "1"))
PW = WMAX // TPP           # partitions per window
WB = P // PW               # windows per 128-partition gather block
NB = NW // WB              # gather instruction count (>= 2)
# mask tensor layout (bf16): iB [0:16] | NB*TPP window mask blocks of
# width WB | bias row0 [JW] | ones row0 [BL]
MSK_B0 = BL + NB * TPP * WB       # bias column offset
MSK_O0 = MSK_B0 + JW              # ones column offset
MSKW = MSK_O0 + BL

_cache: dict = {}


def _consts_cpk():
    """Static part of the mask tensor: identity block + ones row."""
    import ml_dtypes
    msk = np.zeros((P, MSKW), ml_dtypes.bfloat16)
    msk[0:BL, 0:BL] = np.eye(BL, dtype=np.float32)
    msk[0, MSK_O0:MSK_O0 + BL] = 1.0
    return msk


def _build_fast(reps=1, hw_loop=False, num_devices=None):
    import contextlib
    import concourse.bass as bass
    import concourse.tile as tile
    from concourse import bacc, mybir

    f32 = mybir.dt.float32
    bf16 = mybir.dt.bfloat16
    i32 = mybir.dt.int32

    nc = bacc.Bacc("TRN2", target_bir_lowering=False, debug=False,
                   num_devices=num_devices or N_CORES)

    h = nc.dram_tensor("h", [BL * S, H], bf16, kind="ExternalInput")
    idxd = nc.dram_tensor("idx", [P, NB], i32, kind="ExternalInput")
    mskd = nc.dram_tensor("msk", [P, MSKW], bf16, kind="ExternalInput")
    wpk = nc.dram_tensor("wpk", [P, NKT * JW], bf16, kind="ExternalInput")
    out = nc.dram_tensor("out", [BL, JW], bf16, kind="ExternalOutput")

    with tile.TileContext(nc) as tc:
        with (
            tc.tile_pool(name="work", bufs=1) as wpool,
            tc.tile_pool(name="wtiles", bufs=NWC) as wtpool,
            tc.tile_pool(name="psum", bufs=1, space="PSUM") as ppool,
        ):
          loop_ctx = (tc.For_i(0, reps, 1) if hw_loop
                      else contextlib.nullcontext())
          with loop_ctx:
            for _rep in range(1 if hw_loop else reps):
                # --- tiny prefix loads; idx first (gathers wait on it) ---
                idx_t = wpool.tile([P, NB], i32, tag="idx")
                nc.sync.dma_start(idx_t[:], idxd[:, :])
                msk_t = wpool.tile([P, MSKW], bf16, tag="msk")
                nc.sync.dma_start(msk_t[:], mskd[:, :])

                # CLS rows: h[b*S, :] -- fixed-stride DMA on the ACT ring
                # (behind the ~1.3us Tanh table load; not critical-path)
                cls_t = wpool.tile([BL, H], bf16, tag="cls")
                h_bsd = h.ap().rearrange("(b s) d -> b s d", s=S)
                nc.scalar.dma_start(cls_t[:], h_bsd[:, 0, :])

                # --- W chunk 0 streams alongside the gathers ---
                wcs = [wtpool.tile([P, KTC * JW], bf16, tag="wc",
                                   name=f"wc{i}") for i in range(NWC)]
                nc.sync.dma_start(wcs[0][:], wpk[:, 0:KTC * JW])

                # --- the token windows: NB indirect DMAs; each partition
                # pulls TPP consecutive token rows in one contiguous read
                gt = wpool.tile([P, NB * TPP * H], bf16, tag="gt")
                gdmas = []
                for nb in range(NB):
                    gd = nc.gpsimd.indirect_dma_start(
                        out=gt[:, nb * TPP * H:(nb + 1) * TPP * H],
                        out_offset=None,
                        in_=h.ap(),
                        in_offset=bass.IndirectOffsetOnAxis(
                            ap=idx_t[:, nb:nb + 1], axis=0),
                        bounds_check=BL * S - 1,
                        oob_is_err=False,
                    )
                    gdmas.append(gd)

                # --- W chunks 1-3 staggered behind gather chunks 2-4 so
                # the gathers keep >=50% of HBM bandwidth while W streams
                for c4 in range(1, NWC):
                    eng = nc.sync if c4 % 2 == 0 else nc.scalar
                    wd = eng.dma_start(wcs[c4][:],
                                       wpk[:, c4 * KTC * JW:
                                           (c4 + 1) * KTC * JW])
                    if not _os.environ.get("K_NODEP"):
                        gi = min(c4 * NB // NWC, NB - 1)
                        bass._add_dep_helper(wd.ins, gdmas[gi].ins,
                                             sync=True,
                                             reason="stagger W after gathers")

                # --- reduction matmuls -> featsT[3072, BL] in PSUM ---
                # one PSUM tile per segment so each segment's SBUF copy only
                # waits on its own reductions; gather col c covers range
                # s = c // NG (subj, obj), batch group g = c % NG
                fps = [ppool.tile([P, 8 * BL], f32, tag=f"fp{s}",
                                  space="PSUM", name=f"fp{s}")
                       for s in range(3)]
                for ks in range(8):
                    nc.tensor.matmul(out=fps[0][:, ks * BL:(ks + 1) * BL],
                                     lhsT=cls_t[:BL, ks * P:(ks + 1) * P],
                                     rhs=msk_t[0:BL, 0:BL],
                                     start=True, stop=True)
                for c in range(NGT):
                    sg, g = c // NG, c % NG
                    for ks in range(8):
                        col = ks * BL + 4 * g
                        nc.tensor.matmul(
                            out=fps[1 + sg][:, col:col + 4],
                            lhsT=gt[:, c * H + ks * P:c * H + (ks + 1) * P],
                            rhs=msk_t[:, BL + 4 * c:BL + 4 * (c + 1)],
                            start=True, stop=True)
                ft_sb = wpool.tile([P, NKT * BL], bf16, tag="ftsb")
                # per-segment copies so the pooler matmul can start on
                # segment 0 (CLS) while subj/obj reductions are in flight
                for seg in range(3):
                    nc.vector.tensor_copy(
                        ft_sb[:, seg * 8 * BL:(seg + 1) * 8 * BL],
                        fps[seg][:, :])

                # --- pooler matmul: out[BL, JW] = featsT.T @ Wshard + b ---
                # bias rides first (K=1, operands ready early) so the last
                # W chunk's matmuls are the final PSUM writes
                op_t = ppool.tile([BL, JW], f32, tag="op", space="PSUM")
                nc.tensor.matmul(out=op_t[:BL, :],
                                 lhsT=msk_t[0:1, MSK_O0:MSK_O0 + BL],
                                 rhs=msk_t[0:1, MSK_B0:MSK_B0 + JW],
                                 start=True, stop=False)
                for kt in range(NKT):
                    c4, t = divmod(kt, KTC)
                    nc.tensor.matmul(
                        out=op_t[:BL, :],
                        lhsT=ft_sb[:, kt * BL:(kt + 1) * BL],
                        rhs=wcs[c4][:, t * JW:(t + 1) * JW],
                        start=False, stop=(kt == NKT - 1))
                o_sb = wpool.tile([BL, JW], bf16, tag="osb")
                nc.scalar.activation(
                    out=o_sb[:BL, :], in_=op_t[:BL, :],
                    func=mybir.ActivationFunctionType.Tanh)
                nc.sync.dma_start(out[:, :], o_sb[:])

    nc.compile()
    return nc


def _get_nc():
    if "nc" not in _cache:
        _cache["nc"] = _build_fast()
    return _cache["nc"]


def _core_inputs(hidden_states, subj, obj, wt_full, bias_full, consts, c):
    """Build the in_map for core c = bg * GJ + jg."""
    import ml_dtypes
    bg, jg = divmod(c, GJ)
    lo = bg * BL

    # gather indices + window masks: col c = 2g + s covers batches
    # 4g..4g+3, partition p -> batch 4g + p//32, token start + p%32
    q = np.arange(P) // WMAX          # batch-within-group per partition
    j = np.arange(P) % WMAX           # token lane per partition
    idx = np.empty((P, NGT), np.int32)
    msk = np.asarray(consts).copy()
    for g in range(NG):
        for s, rng in enumerate((subj, obj)):
            col = s * NG + g
            bi = lo + 4 * g + q
            start = rng[bi, 0].astype(np.int64)
            length = (rng[bi, 1] - rng[bi, 0]).astype(np.int64)
            raw = (4 * g + q) * S + start + j
            idx[:, col] = np.minimum(raw, BL * S - 1).astype(np.int32)
            w = (j < length) / np.maximum(length, 1)
            m4 = np.zeros((P, 4), np.float32)
            m4[np.arange(P), q] = w
            msk[:, BL + 4 * col:BL + 4 * (col + 1)] = m4

    msk[0, MSK_B0:MSK_B0 + JW] = (
        np.asarray(bias_full).reshape(-1)[jg * JW:(jg + 1) * JW])

    # W shard pre-arranged: wpk[p, kt*JW + j] = W.T[kt*128 + p, jg*JW + j]
    key = ("wpk", jg)
    if key not in _cache:
        wsh = np.asarray(wt_full)[:, jg * JW:(jg + 1) * JW]
        wpk = np.ascontiguousarray(
            wsh.reshape(NKT, P, JW).transpose(1, 0, 2).reshape(P, NKT * JW)
        ).astype(ml_dtypes.bfloat16)
        _cache[key] = wpk

    key_h = ("h", bg)
    if key_h not in _cache:
        _cache[key_h] = np.ascontiguousarray(
            hidden_states[lo:lo + BL].reshape(BL * S, H)
        ).astype(ml_dtypes.bfloat16)
    return {
        "h": _cache[key_h],
        "idx": idx,
        "msk": msk,
        "wpk": _cache[key],
    }


def kernel(hidden_states, subj_range, obj_range, W, b):
    from concourse.bass_utils import run_bass_kernel_spmd

    hidden_states = np.asarray(hidden_states, dtype=np.float32)
    subj = np.asarray(subj_range).astype(np.int64)
    obj = np.asarray(obj_range).astype(np.int64)
    W = np.asarray(W, dtype=np.float32)
    b = np.asarray(b, dtype=np.float32)
    assert hidden_states.shape == (B, S, H)
    assert subj.shape == (B, 2) and obj.shape == (B, 2)

    max_len = max((subj[:, 1] - subj[:, 0]).max(), (obj[:, 1] - obj[:, 0]).max())
    assert max_len <= WMAX, "fast path requires range length <= 32"

    # per-invocation caches (inputs may differ between calls)
    for k in [k for k in _cache if isinstance(k, tuple)]:
        del _cache[k]

    nc = _get_nc()
    consts = _consts_cpk()
    wt_full = np.ascontiguousarray(W.T)            # [3072, 1024]
    bias_full = np.ascontiguousarray(b[None, :])   # [1, 1024]

    in_maps = [_core_inputs(hidden_states, subj, obj, wt_full, bias_full,
                            consts, c) for c in range(N_CORES)]

    res = run_bass_kernel_spmd(nc, in_maps, core_ids=list(range(N_CORES)))
    out = np.empty((B, H), np.float32)
    for c in range(N_CORES):
        bg, jg = divmod(c, GJ)
        out[bg * BL:(bg + 1) * BL, jg * JW:(jg + 1) * JW] = (
            res.results[c]["out"].astype(np.float32))
    return out


# revision 29
# speedup vs baseline: 3.1449x; 1.4908x over previous
"""Trainium2 Bass kernel for a BertPooler-style segment-reduce:

    first = h[:, 0, :]
    subj  = mean(h[b, subj_range[b,0]:subj_range[b,1], :])
    obj   = mean(h[b, obj_range[b,0]:obj_range[b,1], :])
    out   = tanh(concat([first, subj, obj]) @ W.T + b)

Strategy (8 NeuronCores, 4x2 grid: batch-groups x output-column-groups):
  - Core (bg, jg) owns 16 of the 64 batch rows and 512 of the 1024 output
    columns; W is sharded by output column so each core reads half of W.
  - Everything big rides in bf16 (tolerance is 2e-2): hidden state is cast
    on the host, W is cast + pre-arranged on the host so each W chunk DMA
    reads 6KB contiguous per partition.
  - Ranges cover <= 32 tokens; gather indices (start + lane offsets) and
    the 1/len reduction masks are host-precomputed (O(B) metadata, like
    the range packing itself), so the device does: load idx -> NB
    indirect gathers, each partition pulling TPP consecutive token rows
    in one contiguous DMA read (K_TPP env: 1/2/4).
  - Window sums/means are masked matmuls on the TensorEngine producing
    the transposed feature matrix featsT[3072, BL] directly; CLS rows are
    a strided DMA + identity-mask matmuls.
  - The pooler matmul streams the W shard through the PE as the moving
    bf16 operand; bias is a K=1 accumulating f32 matmul; tanh on ACT.
  - W chunk 0 streams concurrently with the gathers to keep the DMA
    engines saturated; chunks 1-3 are explicitly ordered after the last
    gather chunk (manual dep) so the gathers -- whose consumers are the
    long reduction tail -- are not starved by queue round-robin.
"""

import numpy as np

B, S, H = 64, 512, 1024
N_CORES = 8
GJ = 2                     # output-column groups (W shard factor)
GB = N_CORES // GJ         # batch groups
BL = B // GB               # batches per core
NG = BL // 4               # 4-batch gather groups per core
NGT = 2 * NG               # gather index columns (groups x {subj, obj})
JW = H // GJ               # output columns per core
K3 = 3 * H                 # feats dim
P = 128
NKT = K3 // P              # 24 contraction tiles
NWC = 4                    # W chunks
KTC = NKT // NWC           # k-tiles per W chunk
import os as _os
WMAX = 32                  # max range length the fast path supports
NW = 2 * BL                # windows per core (batches x {subj, obj})
# gather layout: TPP consecutive tokens per partition (one 2*TPP KB
# contiguous DRAM read per descriptor); a window spans PW partitions
TPP = int(_os.environ.get("K_TPP", "1"))
PW = WMAX // TPP           # partitions per window
WB = P // PW               # windows per 128-partition gather block
NB = NW // WB              # gather instruction count (>= 2)
# mask tensor layout (bf16): iB [0:16] | NB*TPP window mask blocks of
# width WB | bias row0 [JW] | ones row0 [BL]
MSK_B0 = BL + NB * TPP * WB       # bias column offset
MSK_O0 = MSK_B0 + JW              # ones column offset
MSKW = MSK_O0 + BL

_cache: dict = {}


def _consts_cpk():
    """Static part of the mask tensor: identity block + ones row."""
    import ml_dtypes
    msk = np.zeros((P, MSKW), ml_dtypes.bfloat16)
    msk[0:BL, 0:BL] = np.eye(BL, dtype=np.float32)
    msk[0, MSK_O0:MSK_O0 + BL] = 1.0
    return msk


def _build_fast(reps=1, hw_loop=False, num_devices=None):
    import contextlib
    import concourse.bass as bass
    import concourse.tile as tile
    from concourse import bacc, mybir

    f32 = mybir.dt.float32
    bf16 = mybir.dt.bfloat16
    i32 = mybir.dt.int32

    nc = bacc.Bacc("TRN2", target_bir_lowering=False, debug=False,
                   num_devices=num_devices or N_CORES)

    h = nc.dram_tensor("h", [BL * S, H], bf16, kind="ExternalInput")
    idxd = nc.dram_tensor("idx", [P, NB], i32, kind="ExternalInput")
    mskd = nc.dram_tensor("msk", [P, MSKW], bf16, kind="ExternalInput")
    wpk = nc.dram_tensor("wpk", [P, NKT * JW], bf16, kind="ExternalInput")
    out = nc.dram_tensor("out", [BL, JW], bf16, kind="ExternalOutput")

    with tile.TileContext(nc) as tc:
        with (
            tc.tile_pool(name="work", bufs=1) as wpool,
            tc.tile_pool(name="wtiles", bufs=NWC) as wtpool,
            tc.tile_pool(name="psum", bufs=1, space="PSUM") as ppool,
        ):
          loop_ctx = (tc.For_i(0, reps, 1) if hw_loop
                      else contextlib.nullcontext())
          with loop_ctx:
            for _rep in range(1 if hw_loop else reps):
                # --- tiny prefix loads; idx first (gathers wait on it) ---
                idx_t = wpool.tile([P, NB], i32, tag="idx")
                nc.sync.dma_start(idx_t[:], idxd[:, :])
                msk_t = wpool.tile([P, MSKW], bf16, tag="msk")
                nc.sync.dma_start(msk_t[:], mskd[:, :])

                # CLS rows: h[b*S, :] -- fixed-stride DMA on the ACT ring
                # (behind the ~1.3us Tanh table load; not critical-path)
                cls_t = wpool.tile([BL, H], bf16, tag="cls")
                h_bsd = h.ap().rearrange("(b s) d -> b s d", s=S)
                nc.scalar.dma_start(cls_t[:], h_bsd[:, 0, :])

                # --- W chunk 0 streams alongside the gathers ---
                wcs = [wtpool.tile([P, KTC * JW], bf16, tag="wc",
                                   name=f"wc{i}") for i in range(NWC)]
                nc.sync.dma_start(wcs[0][:], wpk[:, 0:KTC * JW])

                # --- the token windows: NB indirect DMAs; each partition
                # pulls TPP consecutive token rows in one contiguous read
                gt = wpool.tile([P, NB * TPP * H], bf16, tag="gt")
                gdmas = []
                for nb in range(NB):
                    gd = nc.gpsimd.indirect_dma_start(
                        out=gt[:, nb * TPP * H:(nb + 1) * TPP * H],
                        out_offset=None,
                        in_=h.ap(),
                        in_offset=bass.IndirectOffsetOnAxis(
                            ap=idx_t[:, nb:nb + 1], axis=0),
                        bounds_check=BL * S - 1,
                        oob_is_err=False,
                    )
                    gdmas.append(gd)

                # --- W chunks 1-3 staggered behind gather chunks 2-4 so
                # the gathers keep >=50% of HBM bandwidth while W streams
                for c4 in range(1, NWC):
                    eng = nc.sync if c4 % 2 == 0 else nc.scalar
                    wd = eng.dma_start(wcs[c4][:],
                                       wpk[:, c4 * KTC * JW:
                                           (c4 + 1) * KTC * JW])
                    if not _os.environ.get("K_NODEP"):
                        gi = min(c4 * NB // NWC, NB - 1)
                        bass._add_dep_helper(wd.ins, gdmas[gi].ins,
                                             sync=True,
                                             reason="stagger W after gathers")

                # --- reduction matmuls -> featsT[3072, BL] in PSUM ---
                # one PSUM tile per segment so each segment's SBUF copy only
                # waits on its own reductions; gather col c covers range
                # s = c // NG (subj, obj), batch group g = c % NG
                fps = [ppool.tile([P, 8 * BL], f32, tag=f"fp{s}",
                                  space="PSUM", name=f"fp{s}")
                       for s in range(3)]
                for ks in range(8):
                    nc.tensor.matmul(out=fps[0][:, ks * BL:(ks + 1) * BL],
                                     lhsT=cls_t[:BL, ks * P:(ks + 1) * P],
                                     rhs=msk_t[0:BL, 0:BL],
                                     start=True, stop=True)
                for nb in range(NB):
                    sg = nb * WB // BL            # block's range (0/1)
                    boff = nb * WB % BL           # first batch in block
                    for ks in range(8):
                        col = ks * BL + boff
                        for par in range(TPP):
                            m0 = BL + (nb * TPP + par) * WB
                            nc.tensor.matmul(
                                out=fps[1 + sg][:, col:col + WB],
                                lhsT=gt[:, (nb * TPP + par) * H + ks * P:
                                         (nb * TPP + par) * H + (ks + 1) * P],
                                rhs=msk_t[:, m0:m0 + WB],
                                start=(par == 0), stop=(par == TPP - 1))
                ft_sb = wpool.tile([P, NKT * BL], bf16, tag="ftsb")
                # per-segment copies so the pooler matmul can start on
                # segment 0 (CLS) while subj/obj reductions are in flight
                for seg in range(3):
                    nc.vector.tensor_copy(
                        ft_sb[:, seg * 8 * BL:(seg + 1) * 8 * BL],
                        fps[seg][:, :])

                # --- pooler matmul: out[BL, JW] = featsT.T @ Wshard + b ---
                # bias rides first (K=1, operands ready early) so the last
                # W chunk's matmuls are the final PSUM writes
                op_t = ppool.tile([BL, JW], f32, tag="op", space="PSUM")
                nc.tensor.matmul(out=op_t[:BL, :],
                                 lhsT=msk_t[0:1, MSK_O0:MSK_O0 + BL],
                                 rhs=msk_t[0:1, MSK_B0:MSK_B0 + JW],
                                 start=True, stop=False)
                for kt in range(NKT):
                    c4, t = divmod(kt, KTC)
                    nc.tensor.matmul(
                        out=op_t[:BL, :],
                        lhsT=ft_sb[:, kt * BL:(kt + 1) * BL],
                        rhs=wcs[c4][:, t * JW:(t + 1) * JW],
                        start=False, stop=(kt == NKT - 1))
                o_sb = wpool.tile([BL, JW], bf16, tag="osb")
                nc.scalar.activation(
                    out=o_sb[:BL, :], in_=op_t[:BL, :],
                    func=mybir.ActivationFunctionType.Tanh)
                nc.sync.dma_start(out[:, :], o_sb[:])

    nc.compile()
    return nc


def _get_nc():
    if "nc" not in _cache:
        _cache["nc"] = _build_fast()
    return _cache["nc"]


def _core_inputs(hidden_states, subj, obj, wt_full, bias_full, consts, c):
    """Build the in_map for core c = bg * GJ + jg."""
    import ml_dtypes
    bg, jg = divmod(c, GJ)
    lo = bg * BL

    # gather indices + window masks: block nb holds windows
    # w = nb*WB + p//PW (w: s = w//BL range, b = w%BL batch); partition p
    # reads TPP consecutive tokens from start_b + TPP*(p%PW)
    wloc = np.arange(P) // PW         # window-within-block per partition
    pp = np.arange(P) % PW            # partition-within-window
    idx = np.empty((P, NB), np.int32)
    msk = np.asarray(consts).copy()
    ranges = (subj, obj)
    for nb in range(NB):
        w = nb * WB + wloc
        s, b = w // BL, w % BL
        start = np.asarray([ranges[si][lo + bi, 0] for si, bi in zip(s, b)],
                           dtype=np.int64)
        length = np.asarray([ranges[si][lo + bi, 1] - ranges[si][lo + bi, 0]
                             for si, bi in zip(s, b)], dtype=np.int64)
        raw = b * S + start + TPP * pp
        idx[:, nb] = np.minimum(raw, BL * S - TPP).astype(np.int32)
        for par in range(TPP):
            j = TPP * pp + par
            wgt = (j < length) / np.maximum(length, 1)
            m = np.zeros((P, WB), np.float32)
            m[np.arange(P), wloc] = wgt
            c0 = BL + (nb * TPP + par) * WB
            msk[:, c0:c0 + WB] = m

    msk[0, MSK_B0:MSK_B0 + JW] = (
        np.asarray(bias_full).reshape(-1)[jg * JW:(jg + 1) * JW])

    # W shard pre-arranged: wpk[p, kt*JW + j] = W.T[kt*128 + p, jg*JW + j]
    key = ("wpk", jg)
    if key not in _cache:
        wsh = np.asarray(wt_full)[:, jg * JW:(jg + 1) * JW]
        wpk = np.ascontiguousarray(
            wsh.reshape(NKT, P, JW).transpose(1, 0, 2).reshape(P, NKT * JW)
        ).astype(ml_dtypes.bfloat16)
        _cache[key] = wpk

    key_h = ("h", bg)
    if key_h not in _cache:
        _cache[key_h] = np.ascontiguousarray(
            hidden_states[lo:lo + BL].reshape(BL * S, H)
        ).astype(ml_dtypes.bfloat16)
    return {
        "h": _cache[key_h],
        "idx": idx,
        "msk": msk,
        "wpk": _cache[key],
    }


def kernel(hidden_states, subj_range, obj_range, W, b):
    from concourse.bass_utils import run_bass_kernel_spmd

    hidden_states = np.asarray(hidden_states, dtype=np.float32)
    subj = np.asarray(subj_range).astype(np.int64)
    obj = np.asarray(obj_range).astype(np.int64)
    W = np.asarray(W, dtype=np.float32)
    b = np.asarray(b, dtype=np.float32)
    assert hidden_states.shape == (B, S, H)
    assert subj.shape == (B, 2) and obj.shape == (B, 2)

    max_len = max((subj[:, 1] - subj[:, 0]).max(), (obj[:, 1] - obj[:, 0]).max())
    assert max_len <= WMAX, "fast path requires range length <= 32"

    # per-invocation caches (inputs may differ between calls)
    for k in [k for k in _cache if isinstance(k, tuple)]:
        del _cache[k]

    nc = _get_nc()
    consts = _consts_cpk()
    wt_full = np.ascontiguousarray(W.T)            # [3072, 1024]
    bias_full = np.ascontiguousarray(b[None, :])   # [1, 1024]

    in_maps = [_core_inputs(hidden_states, subj, obj, wt_full, bias_full,
                            consts, c) for c in range(N_CORES)]

    res = run_bass_kernel_spmd(nc, in_maps, core_ids=list(range(N_CORES)))
    out = np.empty((B, H), np.float32)
    for c in range(N_CORES):
        bg, jg = divmod(c, GJ)
        out[bg * BL:(bg + 1) * BL, jg * JW:(jg + 1) * JW] = (
            res.results[c]["out"].astype(np.float32))
    return out
